# revision 2
# baseline (speedup 1.0000x reference)
"""Trainium2 Bass kernel for CausalDecayMemory (B=4, T=4096, d=1024).

Math and structural optimizations (banding + projection composition) are
identical to the baseline kernel.py; see its header. V3 restructures for
the PE weight path and PE-stream density:

  * The Tile pipeline splits every matmul into LDWEIGHTS + MATMUL. On HW
    the weight load costs ~cols/1.2GHz and is only partially hidden.
    V3 orders loops so consecutive matmuls share their stationary, then a
    post-build pass deletes the redundant InstLdweights (moving their
    semaphore waits/updates onto the paired InstMatmult, which preserves
    ordering semantics exactly):
      - G projection in {2jc x 2t} blocks (t-pair outer for DMA pacing):
        each wg[ic,jc] stationary serves 2 consecutive 512-wide matmuls.
      - U projection + banded scores FUSED on the xT[ic,kb] stationary:
        3 consecutive matmuls per load.
      - Retrieve: off outer / output-half inner: 2 matmuls per Sw load.
  * retrieve(kb-2) instead of retrieve(kb-1): the Sw[kb] mask-multiply
    (DVE) gets a full kb-iteration of slack instead of stalling the PE.
  * G phase uses all 8 PSUM banks (4 accumulating + 4 draining).
  * DMA issued in consumption order (SP HWDGE ring is FIFO): wg[jc 0-1],
    xT cols 0:1024 ic-major, wg[jc 2-7], xT cols 1024:2048, wu, mask,
    halo columns (only needed by the last kb iteration).
"""

import math

import numpy as np
import ml_dtypes

from concourse import bass, mybir, tile
from concourse.bass_utils import run_bass_kernel_spmd

BF16 = mybir.dt.bfloat16
F32 = mybir.dt.float32

B, T, D = 4, 4096, 1024
P = 128
NI = D // P            # 8 feature chunks
N_CORES = 8
TQ = T // 2            # 2048 query rows per core
NQB = TQ // P          # 16 query blocks
NOFF = 2               # band width in key blocks
HALO = (NOFF - 1) * P  # 128
TK = TQ + HALO         # 2176 key/value rows per core
NKB = TK // P          # 17 key blocks
SBLK = NOFF * P        # 256 score columns per key block

DEDUP_LDW = True


def _split_sync_waits(nc, maxw: int = 1):
    """Split >maxw sem-waits per instruction onto preceding same-engine nops.

    The walrus in this container rejects more than one sync-wait on several
    instruction encodings ("Too many sync wait commands"). Waiting on each
    semaphore in separate instructions immediately before, on the same
    engine, is semantically identical (the engine blocks either way).
    """
    n = 0
    for fn in nc.m.functions:
        for bb in fn.blocks:
            new = []
            for inst in bb.instructions:
                si = getattr(inst, "sync_info", None)
                if si is not None and si.on_wait and len(si.on_wait) > maxw:
                    waits = list(si.on_wait)
                    si.on_wait = waits[:maxw]
                    for j in range(maxw, len(waits), maxw):
                        nop = mybir.InstNoOp(
                            name=f"{inst.name}-ws{j}", ins=[], outs=[]
                        )
                        nop.engine = inst.engine
                        nop.sync_info = mybir.SyncInfo(
                            on_wait=waits[j:j + maxw], on_update=[]
                        )
                        new.append(nop)
                        n += 1
                new.append(inst)
            bb.instructions[:] = new
    return n


def _ap_sig(ap):
    return (ap.memref, ap.offset, str(ap.ap), str(ap.dtype))


def _dedup_ldweights(nc):
    """Delete InstLdweights whose weights AP equals the previous PE weight
    load in the same basic block, with no different load in between.

    The deleted load's sync waits/updates move onto the next instruction
    (its paired InstMatmult), so all ordering constraints are preserved;
    only the redundant array re-load is elided. Stationary SBUF regions are
    written once per kernel body (before their first load), so the array
    contents stay valid across the dedup window; tracking resets at basic
    block boundaries so loop back-edges reload.
    """
    n = 0
    for fn in nc.m.functions:
        for bb in fn.blocks:
            cur = None
            out = []
            pend_w, pend_u = [], []
            for inst in bb.instructions:
                tn = type(inst).__name__
                if tn == "InstLdweights":
                    sig = _ap_sig(inst.ins[0])
                    if sig == cur:
                        si = inst.sync_info
                        if si is not None:
                            pend_w.extend(si.on_wait or [])
                            pend_u.extend(si.on_update or [])
                        n += 1
                        continue  # drop the redundant load
                    cur = sig
                elif tn == "InstMatmult":
                    pass
                if pend_w or pend_u:
                    si = inst.sync_info
                    if si is None:
                        si = mybir.SyncInfo(on_wait=[], on_update=[])
                        inst.sync_info = si
                    si.on_wait = list(pend_w) + list(si.on_wait or [])
                    si.on_update = list(si.on_update or []) + list(pend_u)
                    pend_w, pend_u = [], []
                out.append(inst)
            assert not pend_w and not pend_u
            bb.instructions[:] = out
    return n


def build_kernel(repeat: int = 1, dedup: bool = DEDUP_LDW):
    """Build the per-core Bass program (SPMD; all 8 cores run this)."""
    nc = bass.Bass("TRN2", target_bir_lowering=False)

    xT_d = nc.dram_tensor("xT", [D, TK], BF16, kind="ExternalInput")
    wg_d = nc.dram_tensor("wg", [D, D], BF16, kind="ExternalInput")
    wu_d = nc.dram_tensor("wu", [D, D], BF16, kind="ExternalInput")
    mask_d = nc.dram_tensor("mask", [P, SBLK], F32, kind="ExternalInput")
    y_d = nc.dram_tensor("y", [TQ, D], F32, kind="ExternalOutput")

    with tile.TileContext(nc) as tc:
        with (
            tc.tile_pool(name="big", bufs=1) as big,
            tc.tile_pool(name="wpool", bufs=2) as wpool,
            tc.tile_pool(name="stage", bufs=3) as stage,
            tc.tile_pool(name="pp", bufs=6, space="PSUM") as pp,
            tc.tile_pool(name="pscore", bufs=2, space="PSUM") as pscore,
        ):
            def body(_=None):
                xT = big.tile([P, NI, TK], BF16, tag="xT")
                GT = big.tile([P, NI, TQ], BF16, tag="GT")
                U = big.tile([P, NKB, D], BF16, tag="U")
                Sw = big.tile([P, NKB, SBLK], BF16, tag="Sw")
                mask = big.tile([P, SBLK], F32, tag="mask")

                xTr = xT_d.rearrange("(c p) t -> p c t", p=P)
                wgr = wg_d.rearrange("(c p) j -> p c j", p=P)
                wg_t = wpool.tile([P, NI, D], BF16, tag="w")
                wu_t = wpool.tile([P, NI, D], BF16, tag="w")

                # DMA in consumption order (SP HWDGE ring drains FIFO).
                nc.sync.dma_start(wg_t[:, :, 0:2 * P], wgr[:, :, 0:2 * P])
                nc.sync.dma_start(xT[:, 0:1, 0:512], xTr[:, 0:1, 0:512])
                nc.sync.dma_start(xT[:, 0:1, 512:1024], xTr[:, 0:1, 512:1024])
                for ic in range(1, NI):
                    nc.sync.dma_start(
                        xT[:, ic:ic + 1, 0:1024], xTr[:, ic:ic + 1, 0:1024]
                    )

                # PE warm-up on a zeroed scratch tile: keeps the HAM clock
                # gate open through the head DMA wait (results unused)
                warm = stage.tile([P, 512], BF16, tag="warm")
                nc.gpsimd.memset(warm[:], 0.0)
                for wi in range(8):
                    pw = pp.tile([P, 512], F32, tag="pp", name=f"pw{wi}")
                    nc.tensor.matmul(
                        pw[:], warm[:, 0:P], warm[:], start=True, stop=True
                    )

                for jp in range(1, 4):
                    nc.sync.dma_start(
                        wg_t[:, :, jp * 2 * P:(jp + 1) * 2 * P],
                        wgr[:, :, jp * 2 * P:(jp + 1) * 2 * P],
                    )
                for ic in range(NI):
                    nc.sync.dma_start(
                        xT[:, ic:ic + 1, 1024:2048], xTr[:, ic:ic + 1, 1024:2048]
                    )
                nc.sync.dma_start(wu_t[:], wu_d.rearrange("(c p) o -> p c o", p=P))
                nc.sync.dma_start(mask[:], mask_d[:])
                # halo columns: only needed by the last kb iteration
                nc.sync.dma_start(xT[:, :, TQ:TK], xTr[:, :, TQ:TK])

                # ---- G projection, transposed: GT[j,t] = sum_i A[i,j] xT[i,t]
                # {2jc x 2t} blocks: each wg[ic,jc] stationary serves 2
                # consecutive matmuls; 4 PSUM banks accumulate per block and
                # the other 4 (incl. the pscore pair) drain.
                for tp in range(2):
                    for jp in range(4):
                        pss = [
                            (pp if i < 3 else pscore).tile(
                                [P, 512], F32,
                                tag="pp" if i < 3 else "ps",
                                name=f"gps{tp}{jp}_{i}")
                            for i in range(4)
                        ]
                        for ic in range(NI):
                            for j2 in range(2):
                                jc = jp * 2 + j2
                                for t2 in range(2):
                                    t0 = tp * 1024 + t2 * 512
                                    nc.tensor.matmul(
                                        pss[j2 * 2 + t2][:],
                                        wg_t[:, ic, jc * P:(jc + 1) * P],
                                        xT[:, ic, t0:t0 + 512],
                                        start=(ic == 0),
                                        stop=(ic == NI - 1),
                                    )
                        for j2 in range(2):
                            for t2 in range(2):
                                jc = jp * 2 + j2
                                t0 = tp * 1024 + t2 * 512
                                nc.vector.tensor_copy(
                                    GT[:, jc, t0:t0 + 512], pss[j2 * 2 + t2][:]
                                )

                # ---- fused per-key-block loop: U projection + banded scores
                # share the xT[ic, kb] stationary (3 matmuls per load);
                # retrieve of query block kb-2 is interleaved.
                def u_and_scores(kb):
                    offmax = min(NOFF - 1, kb)
                    offmin = max(0, kb - (NQB - 1))
                    c0 = (NOFF - 1 - offmax) * P
                    c1 = (NOFF - 1 - offmin) * P + P
                    tq0 = (kb - offmax) * P
                    pu = [pp.tile([P, 512], F32, tag="pp", name=f"pu{kb}_{i}")
                          for i in range(2)]
                    ps = pscore.tile([P, SBLK], F32, tag="ps", name=f"psc{kb}")
                    for ic in range(NI):
                        st = xT[:, ic, kb * P:(kb + 1) * P]
                        for oh in range(2):
                            nc.tensor.matmul(
                                pu[oh][:],
                                st,
                                wu_t[:, ic, oh * 512:(oh + 1) * 512],
                                start=(ic == 0),
                                stop=(ic == NI - 1),
                            )
                        nc.tensor.matmul(
                            ps[:, c0:c1],
                            st,
                            GT[:, ic, tq0:tq0 + (c1 - c0)],
                            start=(ic == 0),
                            stop=(ic == NI - 1),
                        )
                    for oh in range(2):
                        nc.vector.tensor_copy(
                            U[:, kb, oh * 512:(oh + 1) * 512], pu[oh][:]
                        )
                    nc.vector.tensor_mul(
                        Sw[:, kb, c0:c1], ps[:, c0:c1], mask[:, c0:c1]
                    )

                def retrieve(qb):
                    # y[tq, o] = sum_off Sw[:, qb+off].T @ U[qb+off]
                    # off outer / oh inner: one Sw stationary per 2 matmuls.
                    yo = stage.tile([P, D], F32, tag="yo", name=f"yo{qb}")
                    po = [pp.tile([P, 512], F32, tag="pp", name=f"po{qb}_{i}")
                          for i in range(2)]
                    for off in range(NOFF):
                        kb = qb + off
                        st = Sw[:, kb, (NOFF - 1 - off) * P:(NOFF - off) * P]
                        for oh in range(2):
                            nc.tensor.matmul(
                                po[oh][:],
                                st,
                                U[:, kb, oh * 512:(oh + 1) * 512],
                                start=(off == 0),
                                stop=(off == NOFF - 1),
                            )
                    for oh in range(2):
                        nc.vector.tensor_copy(
                            yo[:, oh * 512:(oh + 1) * 512], po[oh][:]
                        )
                    nc.sync.dma_start(y_d[qb * P:(qb + 1) * P, :], yo[:])

                for kb in range(NKB):
                    u_and_scores(kb)
                    if kb >= 2:
                        retrieve(kb - 2)
                retrieve(NQB - 1)

            if repeat > 1:
                hints = (
                    mybir.EngineType.PE,
                    mybir.EngineType.SP,
                    mybir.EngineType.DVE,
                )
                with tc.For_i(0, repeat, 1, hint_engines=hints) as _i:
                    body()
            else:
                body()

    if dedup:
        _dedup_ldweights(nc)
    _split_sync_waits(nc)
    return nc


def _host_inputs(x, Wq, Wk, Wv, Wo, decay_logit, out_scale):
    """Per-core input maps: compose projections, shard x, transpose+cast."""
    x = np.asarray(x, dtype=np.float32)
    decay = float(1.0 / (1.0 + math.exp(-float(np.asarray(decay_logit)))))
    scale = 1.0 / math.sqrt(D)

    bf = ml_dtypes.bfloat16
    A = np.asarray(Wq, np.float64).T @ np.asarray(Wk, np.float64)
    C = (float(np.asarray(out_scale)) * np.asarray(Wo, np.float64)) @ np.asarray(
        Wv, np.float64
    )
    wg = np.ascontiguousarray(A).astype(bf)            # [i, j]
    wu = np.ascontiguousarray(C.T).astype(bf)          # [i, o]

    pp_, qq = np.meshgrid(np.arange(P), np.arange(P), indexing="ij")
    mask = np.zeros((P, SBLK), np.float32)
    for off in range(NOFF):
        expo = off * P + pp_ - qq - 1.0
        blk = np.where(expo >= 0.0, decay ** expo, 0.0) * scale
        mask[:, (NOFF - 1 - off) * P:(NOFF - off) * P] = blk.astype(np.float32)

    in_maps = []
    for c in range(N_CORES):
        b, h = divmod(c, 2)
        t0 = h * TQ
        rows = min(TK, T - t0)
        xs = np.zeros((TK, D), np.float32)
        xs[:rows] = x[b, t0:t0 + rows]
        xT = np.ascontiguousarray(xs.T).astype(bf)
        in_maps.append({"xT": xT, "wg": wg, "wu": wu, "mask": mask})
    return in_maps


_NC_CACHE = {}


def get_nc(repeat: int = 1):
    if repeat not in _NC_CACHE:
        _NC_CACHE[repeat] = build_kernel(repeat)
    return _NC_CACHE[repeat]


def kernel(x, Wq, Wk, Wv, Wo, decay_logit, out_scale):
    nc = get_nc(1)
    in_maps = _host_inputs(x, Wq, Wk, Wv, Wo, decay_logit, out_scale)
    try:
        res = run_bass_kernel_spmd(nc, in_maps, list(range(N_CORES)))
    except Exception:
        # transient NRT device errors have been observed; retry once
        res = run_bass_kernel_spmd(nc, in_maps, list(range(N_CORES)))
    y = np.empty((B, T, D), np.float32)
    for c in range(N_CORES):
        b, h = divmod(c, 2)
        y[b, h * TQ:(h + 1) * TQ, :] = res.results[c]["y"]
    return y
